# revision 61
# baseline (speedup 1.0000x reference)
"""Trainium2 Bass kernel for location-sensitive attention (nn_AttentionMechanism).

Full-input contract: kernel(**inputs) takes the unsharded inputs (B=32), shards
batch-wise across 8 NeuronCores (4 batches per core), runs one SPMD Bass/Tile
program per core, and gathers the full outputs.

Math per batch b (L=2048, KD=QD=AD=512, C=10, taps=201):
    k_proj = key @ w_key + b_key                    # [L, A]
    q_proj = query @ w_query                        # [1, A]
    c_proj = conv1d(aw, conv_w) @ w_conv            # [L, A]
    e      = v . tanh(k_proj + q_proj + c_proj)     # [L]
    e      = where(arange(L) < klen, e, -1024)
    aw_new = softmax(e)                             # [L]
    cv     = aw_new @ value                         # [V]

On-chip layout: A on partitions, L on the free dim, so the whole pre-tanh sum
is accumulated in PSUM by matmuls (keyT from PE transposes; conv folded to a
single rank-10 matrix W2 = w_conv^T @ conv_w applied to an im2col of aw), the
q_proj+b_key add rides the ScalarE tanh bias, and the v-dot and aw@value
contractions are matmuls as well.
"""

import numpy as np

# problem constants (hardcoded per the self-contained-kernel contract)
B_FULL = 32
N_CORES = 8
B = B_FULL // N_CORES  # batches per core
L = 2048
KD = 512
AD = 512
V = 512
C_OUT = 10
TAPS = 201
PAD = 100
LPAD = L + 2 * PAD
NEG = -1024.0

P = 128
NKD = KD // P       # kd chunks
NA = AD // P        # a chunks
LCHUNK = 512
NLC = L // LCHUNK   # l chunks
NLT = LCHUNK // P   # l tiles per chunk
NVT = L // P        # value tiles (within chunk lc: l = 512*lc + 4*p + j)

_CACHE = {}


def _build_nc(
    f32r_mm: bool = True,
    f32r_tr: bool = False,
):
    import concourse.bacc as bacc
    import concourse.bass as bass
    import concourse.tile as tile
    from concourse import mybir
    from concourse.masks import make_identity
    from contextlib import ExitStack

    f32 = mybir.dt.float32
    f32r = mybir.dt.float32r
    bf16 = mybir.dt.bfloat16
    i32 = mybir.dt.int32
    AF = mybir.ActivationFunctionType
    ALU = mybir.AluOpType

    # dtype for fast-matmul operands: float32r runs the PE at bf16 rate for
    # N>=256 (vs 4 cyc/row for plain fp32). Tiles feeding f32r matmuls are
    # allocated as f32r; engine producers round into them, DMA producers
    # bitcast (walrus only enforces rounding on engine producers).
    mdt = f32r if f32r_mm else f32

    def mcast(ap):
        return ap.bitcast(mdt)

    def trcast(ap):
        return ap.bitcast(f32r) if f32r_tr else ap

    nc = bacc.Bacc("TRN2", target_bir_lowering=False, debug=False)

    key_h = nc.dram_tensor("key", [B, L, KD], f32, kind="ExternalInput")
    klens_h = nc.dram_tensor("klens", [B], i32, kind="ExternalInput")
    value_h = nc.dram_tensor("value", [B, L, V], f32, kind="ExternalInput")
    query_h = nc.dram_tensor("query", [B, KD], f32, kind="ExternalInput")
    aw_h = nc.dram_tensor("aw", [B, L], f32, kind="ExternalInput")
    wkey_h = nc.dram_tensor("w_key", [KD, AD], f32, kind="ExternalInput")
    bkey_h = nc.dram_tensor("b_key", [AD], f32, kind="ExternalInput")
    wquery_h = nc.dram_tensor("w_query", [KD, AD], f32, kind="ExternalInput")
    wconv_h = nc.dram_tensor("w_conv", [C_OUT, AD], f32, kind="ExternalInput")
    convw_h = nc.dram_tensor("conv_w", [C_OUT, TAPS], f32, kind="ExternalInput")
    v_h = nc.dram_tensor("v", [AD], f32, kind="ExternalInput")

    cv_h = nc.dram_tensor("cv_out", [B, V], f32, kind="ExternalOutput")
    awn_h = nc.dram_tensor("aw_out", [B, L], f32, kind="ExternalOutput")

    awpad_h = nc.dram_tensor("awpad_scratch", [B, LPAD], bf16)

    JCS = ((0, P), (1, TAPS - P))  # conv tap chunks: 128 + 73

    with tile.TileContext(nc) as tc:
        with ExitStack() as ctx:
            consts = ctx.enter_context(tc.tile_pool(name="consts", bufs=1))
            kpool = ctx.enter_context(tc.tile_pool(name="kpool", bufs=3))
            ktpool = ctx.enter_context(tc.tile_pool(name="ktpool", bufs=2))
            tpool = ctx.enter_context(tc.tile_pool(name="tpool", bufs=6))
            impool = ctx.enter_context(tc.tile_pool(name="impool", bufs=2))
            vpool = ctx.enter_context(tc.tile_pool(name="vpool", bufs=2))
            rows = ctx.enter_context(tc.tile_pool(name="rows", bufs=2))
            smalls = ctx.enter_context(tc.tile_pool(name="smalls", bufs=2))

            # key prefetch: issue the DMA for step s while computing step s-1
            def key_dma(b, lc):
                k_sb = kpool.tile([P, NLT * KD], f32, name="k_sb", tag="k_sb")
                nc.sync.dma_start(
                    out=k_sb,
                    in_=bass.AP(
                        tensor=key_h,
                        offset=(b * L + lc * LCHUNK) * KD,
                        ap=[[KD, P], [P * KD, NLT], [1, KD]],
                    ),
                )
                return k_sb

            # im2col on the SWDGE rail: im[jc][j, l] = awpad[b, jc*128 + j + l]
            def im2col_dma(b):
                ims = []
                for jc, sz in JCS:
                    im = impool.tile([P, L], bf16, name=f"im{jc}", tag=f"im{jc}")
                    nc.gpsimd.dma_start(
                        out=im[:sz, :],
                        in_=bass.AP(
                            tensor=awpad_h,
                            offset=b * LPAD + jc * P,
                            ap=[[1, sz], [1, L]],
                        ),
                    )
                    ims.append(im)
                return ims

            # ---------------- prologue ----------------
            with tc.tile_pool(name="ppsum", bufs=1, space="PSUM") as ppsum:
                ident = consts.tile([P, P], f32)
                make_identity(nc, ident)

                # aw chain early, entirely on the SWDGE rail: zero-padded bf16
                # aw row round-tripped to DRAM so the im2col DMAs can use
                # element-granular sliding windows
                with tc.tile_pool(name="prosb", bufs=1) as prosb:
                    awrow = prosb.tile([B, LPAD], bf16)
                    nc.vector.memset(awrow, 0.0)
                    nc.gpsimd.dma_start(out=awrow[:, PAD:PAD + L], in_=aw_h.ap())
                    awpad_inst = nc.gpsimd.dma_start(out=awpad_h.ap(), in_=awrow)

                # first key chunk ahead of everything (each DMA ahead of it
                # costs ~620ns of serialized HWDGE descriptor generation)
                k_pending = key_dma(0, 0)
                ims0 = im2col_dma(0)

                # tiny contiguous loads (conv weights, query, klens)
                cw = consts.tile([C_OUT, TAPS], f32)
                nc.sync.dma_start(out=cw, in_=convw_h.ap())
                wc_r = consts.tile([C_OUT, AD], mdt)
                nc.sync.dma_start(out=wc_r, in_=mcast(wconv_h.ap()))
                query_sb = consts.tile([B, KD], f32)
                nc.sync.dma_start(out=query_sb, in_=query_h.ap())
                kl_i = consts.tile([1, B], i32)
                nc.sync.dma_start(out=kl_i, in_=klens_h.ap().unsqueeze(0))
                kl_f = consts.tile([1, B], f32)
                nc.vector.tensor_copy(out=kl_f, in_=kl_i)

                wk = []
                for c in range(NKD):
                    wkc = consts.tile([P, AD], mdt, name=f"wk{c}")
                    nc.sync.dma_start(
                        out=wkc, in_=mcast(wkey_h.ap()[c * P:(c + 1) * P, :])
                    )
                    wk.append(wkc)

                # cwT[j, c] = conv_w[c, j] (bf16, stage-1 lhsT) and
                # qT[c] = query chunk transposed — both via PE transpose
                # (element-granular gather DMAs would serialize ~0.2us each
                # on the DMA engines right when key00/wk need them)
                cwT = []
                for jc, sz in JCS:
                    cwp = ppsum.tile([P, C_OUT], f32, name=f"cwp{jc}", tag=f"cwp{jc}")
                    nc.tensor.transpose(
                        cwp[:sz, :], cw[:, jc * P:jc * P + sz],
                        ident[:C_OUT, :C_OUT],
                    )
                    cwTc = consts.tile([P, C_OUT], bf16, name=f"cwT{jc}")
                    nc.any.tensor_copy(out=cwTc[:sz, :], in_=cwp[:sz, :])
                    cwT.append(cwTc)
                qT = []
                for c in range(NKD):
                    qTp = ppsum.tile([P, B], f32, name=f"qTp{c}", tag=f"qTp{c}")
                    nc.tensor.transpose(
                        qTp, query_sb[:, c * P:(c + 1) * P], ident[:B, :B]
                    )
                    qTc = consts.tile([P, B], f32, name=f"qT{c}")
                    nc.any.tensor_copy(out=qTc, in_=qTp)
                    qT.append(qTc)

                # strided gathers + the late-needed wq after the key/wk loads
                bk = consts.tile([P, NA], f32)
                nc.sync.dma_start(
                    out=bk, in_=bass.AP(tensor=bkey_h, offset=0, ap=[[1, P], [P, NA]])
                )
                v_sb = consts.tile([P, NA], mdt)
                nc.sync.dma_start(
                    out=v_sb,
                    in_=mcast(bass.AP(tensor=v_h, offset=0, ap=[[1, P], [P, NA]])),
                )
                k_pending2 = key_dma(0, 1)
                wq = []
                for c in range(NKD):
                    wqc = consts.tile([P, AD], f32, name=f"wq{c}")
                    nc.sync.dma_start(out=wqc, in_=wquery_h.ap()[c * P:(c + 1) * P, :])
                    wq.append(wqc)


            # ---------------- main psum pools ----------------
            trpsum = ctx.enter_context(tc.tile_pool(name="trpsum", bufs=1, space="PSUM"))
            spsum = ctx.enter_context(tc.tile_pool(name="spsum", bufs=3, space="PSUM"))
            cfpsum = ctx.enter_context(tc.tile_pool(name="cfpsum", bufs=1, space="PSUM"))
            epsum = ctx.enter_context(tc.tile_pool(name="epsum", bufs=1, space="PSUM"))
            cvpsum = ctx.enter_context(tc.tile_pool(name="cvpsum", bufs=1, space="PSUM"))

            # f32 iota is exact for 0..L-1; emitted late so the in-order Pool
            # engine runs the identity/aw/im2col chain first
            iota_f = consts.tile([1, L], f32)
            iota_inst = nc.gpsimd.iota(
                iota_f, pattern=[[1, L]], base=0, channel_multiplier=0,
                allow_small_or_imprecise_dtypes=True,
            )
            # keep the 3us iota behind the identity/aw chain on the in-order
            # Pool engine (it is only needed by the first mask op at ~11us)
            tile.add_dep_helper(iota_inst.ins, awpad_inst.ins, sync=False,
                                reason="iota after aw chain")

            # bias[a, (c, b)] = q_projT[a, b] + b_key[a]. Emitted here (and
            # priority-demoted) because wq arrives late and engines run
            # in-order: these matmuls must not stall batch-0 work behind them.
            # The qp tiles share the cf psum slot to stay inside 8 banks.
            bias_sb = consts.tile([P, NA * B], f32)
            with tc.high_priority(offset=-80):
                for c in range(NA):
                    qp = cfpsum.tile([P, B], f32, name="qp", tag="cfp")
                    for kc in range(NKD):
                        nc.tensor.matmul(
                            out=qp, lhsT=wq[kc][:, c * P:(c + 1) * P], rhs=qT[kc],
                            start=(kc == 0), stop=(kc == NKD - 1),
                        )
                    nc.vector.tensor_scalar_add(
                        out=bias_sb[:, c * B:(c + 1) * B], in0=qp,
                        scalar1=bk[:, c:c + 1],
                    )

            tq = NVT // NLC  # value t-tiles loaded per lc
            for b in range(B):
                # value tiles (l = 128t + p): vt[p, (t, v)] = value[b, 128t+p, v];
                # loaded a quarter per lc so key loads aren't stuck behind them
                vtile = vpool.tile([P, NVT * V], mdt, name="vtile", tag="vtile")
                vts = [vtile[:, t * V:(t + 1) * V] for t in range(NVT)]

                ims = ims0 if b == 0 else im2col_dma(b)

                exp_b = rows.tile([1, L], f32, name="exp_b", tag="exp_b")
                sums4 = smalls.tile([1, NLC], f32, name="sums4", tag="sums4")
                awT = smalls.tile([P, NVT], mdt, name="awT", tag="awT")
                cvp = cvpsum.tile([1, V], f32, name="cvp", tag="cvp")
                for lc in range(NLC):
                    k_sb = k_pending
                    k_pending = k_pending2
                    step = b * NLC + lc + 2
                    if step < B * NLC:
                        k_pending2 = key_dma(step // NLC, step % NLC)
                    else:
                        k_pending2 = None
                    kts = []
                    for kc in range(NKD):
                        trp = trpsum.tile([P, LCHUNK], f32, name="trp", tag=f"trp{kc % 2}")
                        for i in range(NLT):
                            nc.tensor.transpose(
                                trcast(trp[:, i * P:(i + 1) * P]),
                                trcast(k_sb[:, i * KD + kc * P:i * KD + (kc + 1) * P]),
                                trcast(ident),
                            )
                        kt = ktpool.tile([P, LCHUNK], mdt, name="kt", tag=f"kt{kc}")
                        nc.any.tensor_copy(out=kt, in_=trp)
                        kts.append(kt)

                    # value t-tile (t = lc*4 + j) holds rows l = 512lc + 4p + j
                    nc.sync.dma_start(
                        out=vtile[:, lc * tq * V:(lc + 1) * tq * V],
                        in_=mcast(bass.AP(
                            tensor=value_h,
                            offset=(b * L + lc * LCHUNK) * V,
                            ap=[[tq * V, P], [V, tq], [1, V]],
                        )),
                    )
                    # conv stage 1: cf[c, l] = sum_j conv_w[c, j] awpad[l + j]
                    cfp = cfpsum.tile([C_OUT, LCHUNK], f32, name="cfp", tag="cfp")
                    for jc, sz in JCS:
                        nc.tensor.matmul(
                            out=cfp,
                            lhsT=cwT[jc][:sz, :],
                            rhs=ims[jc][:sz, lc * LCHUNK:(lc + 1) * LCHUNK],
                            start=(jc == 0), stop=(jc == 1),
                        )
                    cf_sb = smalls.tile([C_OUT, LCHUNK], mdt, name="cf_sb", tag="cf_sb")
                    nc.any.tensor_copy(out=cf_sb, in_=cfp)

                    ep = epsum.tile([1, LCHUNK], f32, name="ep", tag="ep")
                    for a in range(NA):
                        sp = spsum.tile([P, LCHUNK], f32, name="sp", tag="sp")
                        for kc in range(NKD):
                            nc.tensor.matmul(
                                out=sp,
                                lhsT=wk[kc][:, a * P:(a + 1) * P],
                                rhs=kts[kc],
                                start=(kc == 0), stop=False,
                            )
                        nc.tensor.matmul(
                            out=sp,
                            lhsT=wc_r[:, a * P:(a + 1) * P],
                            rhs=cf_sb,
                            start=False, stop=True,
                        )
                        tt = tpool.tile([P, LCHUNK], mdt, name="tt", tag="tt")
                        nc.scalar.activation(
                            out=tt, in_=sp, func=AF.Tanh,
                            bias=bias_sb[:, a * B + b:a * B + b + 1], scale=1.0,
                        )
                        nc.tensor.matmul(
                            out=ep,
                            lhsT=v_sb[:, a:a + 1],
                            rhs=tt,
                            start=(a == 0), stop=(a == NA - 1),
                        )
                    # per-chunk softmax numerator: exp straight out of PSUM
                    # (no max-subtraction needed: |e| <= sum|v| ~ 8, so exp
                    # never overflows), then mask + partial sum in one DVE op:
                    # exp_b = (iota < klen_b) * exp(e), sums4[lc] = sum(chunk)
                    sl = slice(lc * LCHUNK, (lc + 1) * LCHUNK)
                    nc.scalar.activation(out=exp_b[:, sl], in_=ep, func=AF.Exp)
                    nc.vector.scalar_tensor_tensor(
                        out=exp_b[:, sl], in0=iota_f[:, sl],
                        scalar=kl_f[:, b:b + 1], in1=exp_b[:, sl],
                        op0=ALU.is_lt, op1=ALU.mult,
                        accum_out=sums4[:, lc:lc + 1],
                    )
                    # awT[p, lc*4+j] = exp_b[512lc + 4p + j] matching the value
                    # tiling, then the chunk's share of the aw @ value matmuls
                    nc.sync.dma_start(
                        out=awT[:, lc * tq:(lc + 1) * tq],
                        in_=mcast(
                            exp_b[:, sl].rearrange("o (p j) -> o p j", j=tq)
                        ),
                    )
                    for j in range(tq):
                        t = lc * tq + j
                        nc.tensor.matmul(
                            out=cvp,
                            lhsT=awT[:, t:t + 1],
                            rhs=vts[t],
                            start=(t == 0), stop=(t == NVT - 1),
                        )

                sum_b = smalls.tile([1, 1], f32, name="sum_b", tag="sum_b")
                nc.vector.reduce_sum(out=sum_b, in_=sums4, axis=mybir.AxisListType.X)
                rec_b = smalls.tile([1, 1], f32, name="rec_b", tag="rec_b")
                nc.vector.reciprocal(out=rec_b, in_=sum_b)

                cv_sb = smalls.tile([1, V], f32, name="cv_sb", tag="cv_sb")
                nc.vector.tensor_scalar_mul(out=cv_sb, in0=cvp, scalar1=rec_b)
                nc.sync.dma_start(out=cv_h.ap()[b:b + 1, :], in_=cv_sb)

                awn_b = rows.tile([1, L], f32, name="awn_b", tag="awn_b", bufs=1)
                nc.vector.tensor_scalar_mul(out=awn_b, in0=exp_b, scalar1=rec_b)
                nc.sync.dma_start(out=awn_h.ap()[b:b + 1, :], in_=awn_b)

    nc.compile()
    return nc


def get_nc(**kwargs):
    key = tuple(sorted(kwargs.items()))
    if key not in _CACHE:
        _CACHE[key] = _build_nc(**kwargs)
    return _CACHE[key]


def make_in_maps(inputs):
    key = np.ascontiguousarray(np.asarray(inputs["key"]), dtype=np.float32)
    klens = np.ascontiguousarray(np.asarray(inputs["klens"]), dtype=np.int32)
    value = np.ascontiguousarray(np.asarray(inputs["value"]), dtype=np.float32)
    query = np.ascontiguousarray(
        np.asarray(inputs["query"]).reshape(B_FULL, KD), dtype=np.float32
    )
    aw = np.ascontiguousarray(
        np.asarray(inputs["aw"]).reshape(B_FULL, L), dtype=np.float32
    )
    w_key = np.ascontiguousarray(np.asarray(inputs["w_key"]), dtype=np.float32)
    b_key = np.ascontiguousarray(np.asarray(inputs["b_key"]), dtype=np.float32)
    w_query = np.ascontiguousarray(np.asarray(inputs["w_query"]), dtype=np.float32)
    w_conv = np.ascontiguousarray(np.asarray(inputs["w_conv"]), dtype=np.float32)
    conv_w = np.ascontiguousarray(
        np.asarray(inputs["conv_w"]).reshape(C_OUT, TAPS), dtype=np.float32
    )
    v = np.ascontiguousarray(np.asarray(inputs["v"]), dtype=np.float32)

    in_maps = []
    for c in range(N_CORES):
        sl = slice(c * B, (c + 1) * B)
        in_maps.append({
            "key": key[sl], "klens": klens[sl], "value": value[sl],
            "query": query[sl], "aw": aw[sl],
            "w_key": w_key, "b_key": b_key, "w_query": w_query,
            "w_conv": w_conv, "conv_w": conv_w, "v": v,
        })
    return in_maps


def kernel(**inputs):
    from concourse.bass_utils import run_bass_kernel_spmd

    nc = get_nc()
    in_maps = make_in_maps(inputs)
    res = run_bass_kernel_spmd(nc, in_maps, core_ids=list(range(N_CORES)))
    cv = np.concatenate(
        [res.results[c]["cv_out"] for c in range(N_CORES)], axis=0
    ).reshape(B_FULL, 1, V)
    awn = np.concatenate(
        [res.results[c]["aw_out"] for c in range(N_CORES)], axis=0
    ).reshape(B_FULL, L, 1)
    return cv, awn
